# revision 7
# baseline (speedup 1.0000x reference)
"""Trainium2 Bass kernel for ExpandFormerV16 (masked multi-domain MLP over embeddings).

Reference computation:
    h    = embed[x]                                   # [B,S,512]
    mask = token_mask[x]                              # [B,S,16]
    act  = gelu(einsum('bsD,nDd->bsnd', h, W1))       # exact (erf) gelu
    corr = 0.1 * einsum('bsnd,bsn,ndD->bsD', act, mask, W2)
    out  = h + corr

Strategy: data-parallel over the 16384 tokens -> 2048 tokens per core on 8
cores; embedding table and domain weights replicated. Per core:
  - indirect-DMA gather of embed rows (h stays exact fp32 for the final add)
  - PE transpose h -> hT [D, tok]
  - GEMM1 (f32r): actT[n] = W1[n].T @ hT, K=512 accumulated in PSUM
  - exact-erf Gelu on ACT engine, mask multiply on DVE (mask rows broadcast
    across partitions with a stride-0-partition DMA)
  - GEMM2 (f32r): corr[tok,:] += actmT[n].T @ (0.1*W2[n]) over all 16 domains
  - DVE add h + corr, DMA out
"""

import numpy as np

import concourse.bacc as bacc
import concourse.bass as bass
import concourse.tile as tile
from concourse import mybir
from concourse.bass import IndirectOffsetOnAxis
from concourse.bass_utils import run_bass_kernel_spmd
from concourse.masks import make_identity

# Problem shapes (hardcoded per contest contract)
VOCAB, D, ND, DD = 32000, 512, 16, 128
B, S = 8, 2048
N_CORES = 8
T = (B * S) // N_CORES          # tokens per core = 2048
P = 128                         # partitions
KCH = D // P                    # 4 contraction chunks of 128
TBLK = 512                      # tokens per processing block (PSUM free dim)
NBLK = T // TBLK                # 4 blocks per core
JT = TBLK // P                  # 4 token-tiles of 128 per block

F32 = mybir.dt.float32
F32R = mybir.dt.float32r
I32 = mybir.dt.int32

_CACHE: dict = {}


def _build_program():
    nc = bacc.Bacc(
        "TRN2",
        target_bir_lowering=False,
        debug=False,
        enable_asserts=False,
        num_devices=N_CORES,
    )

    idx_d = nc.dram_tensor("idx", [P, T // P], I32, kind="ExternalInput")
    embed_d = nc.dram_tensor("embed", [VOCAB, D], F32, kind="ExternalInput")
    w1_d = nc.dram_tensor("w1", [P, KCH, ND, DD], F32R, kind="ExternalInput")
    w2_d = nc.dram_tensor("w2", [P, ND, D], F32R, kind="ExternalInput")
    maskt_d = nc.dram_tensor("maskt", [ND, T], F32, kind="ExternalInput")
    out_d = nc.dram_tensor("out", [T, D], F32, kind="ExternalOutput")

    with tile.TileContext(nc) as tc:
        with (
            tc.tile_pool(name="consts", bufs=1) as consts,
            tc.tile_pool(name="hpool", bufs=2) as hpool,
            tc.tile_pool(name="htpool", bufs=2) as htpool,
            tc.tile_pool(name="gpool", bufs=3) as gpool,
            tc.tile_pool(name="mpool", bufs=3) as mpool,
            tc.tile_pool(name="ampool", bufs=2) as ampool,
            tc.tile_pool(name="opool", bufs=3) as opool,
            tc.tile_pool(name="tpsum", bufs=2, space="PSUM") as tpsum,
            tc.tile_pool(name="apsum", bufs=2, space="PSUM") as apsum,
            tc.tile_pool(name="cpsum", bufs=2, space="PSUM") as cpsum,
        ):
            # --- constants ---
            idx_sb = consts.tile([P, T // P], I32)
            nc.sync.dma_start(idx_sb[:], idx_d.ap())
            w1_sb = consts.tile([P, KCH, ND, DD], F32R)
            nc.sync.dma_start(w1_sb[:], w1_d.ap())
            w2_sb = consts.tile([P, ND, D], F32R)
            nc.sync.dma_start(w2_sb[:], w2_d.ap())
            ident = consts.tile([P, P], F32)
            make_identity(nc, ident[:])

            for blk in range(NBLK):
                # --- gather embed rows for this block's 4 token-tiles ---
                h_blk = hpool.tile([P, JT, D], F32)
                for j in range(JT):
                    t = blk * JT + j
                    nc.gpsimd.indirect_dma_start(
                        out=h_blk[:, j, :],
                        out_offset=None,
                        in_=embed_d.ap(),
                        in_offset=IndirectOffsetOnAxis(
                            ap=idx_sb[:, t : t + 1], axis=0
                        ),
                    )

                # --- transpose h [tok, D] -> hT [D, tok] via PE ---
                hT_blk = htpool.tile([P, KCH, TBLK], F32R)
                for j in range(JT):
                    for k in range(KCH):
                        tp = tpsum.tile([P, P], F32)
                        nc.tensor.transpose(
                            tp[:], h_blk[:, j, k * P : (k + 1) * P], ident[:]
                        )
                        nc.vector.tensor_copy(
                            hT_blk[:, k, j * P : (j + 1) * P], tp[:]
                        )

                # --- GEMM1 + gelu + mask per domain ---
                actm_blk = ampool.tile([P, ND, TBLK], F32R)
                for n in range(ND):
                    act_ps = apsum.tile([P, TBLK], F32)
                    for k in range(KCH):
                        nc.tensor.matmul(
                            act_ps[:],
                            lhsT=w1_sb[:, k, n, :],
                            rhs=hT_blk[:, k, :],
                            start=(k == 0),
                            stop=(k == KCH - 1),
                        )
                    actg = gpool.tile([P, TBLK], F32)
                    nc.scalar.activation(
                        actg[:], act_ps[:], mybir.ActivationFunctionType.Gelu
                    )
                    # mask row [1, TBLK] broadcast to all 128 partitions via
                    # stride-0 partition DMA (groupnorm bias pattern)
                    m_tile = mpool.tile([P, TBLK], F32)
                    m_src = bass.AP(
                        tensor=maskt_d.ap().tensor,
                        offset=n * T + blk * TBLK,
                        ap=[[0, P], [1, TBLK]],
                    )
                    nc.gpsimd.dma_start(out=m_tile[:], in_=m_src)
                    nc.vector.tensor_mul(actm_blk[:, n, :], actg[:], m_tile[:])

                # --- GEMM2: corr[tok, D] accumulated over domains ---
                for j in range(JT):
                    corr_ps = cpsum.tile([P, D], F32)
                    for n in range(ND):
                        nc.tensor.matmul(
                            corr_ps[:],
                            lhsT=actm_blk[:, n, j * P : (j + 1) * P],
                            rhs=w2_sb[:, n, :],
                            start=(n == 0),
                            stop=(n == ND - 1),
                        )
                    out_sb = opool.tile([P, D], F32)
                    nc.vector.tensor_add(out_sb[:], corr_ps[:], h_blk[:, j, :])
                    row0 = (blk * JT + j) * P
                    nc.sync.dma_start(
                        out=out_d.ap()[row0 : row0 + P, :], in_=out_sb[:]
                    )

    nc.compile()
    return nc


def _prep_inputs(x, embed, W1, W2, token_mask):
    """Host-side shard + layout prep. Returns per-core in_maps."""
    xf = np.ascontiguousarray(x.reshape(-1).astype(np.int32))
    w1h = np.ascontiguousarray(
        W1.astype(np.float32)
        .transpose(1, 0, 2)          # [512, 16, 128]
        .reshape(KCH, P, ND, DD)     # [k, p, n, d]
        .transpose(1, 0, 2, 3)       # [p, k, n, d]
    )
    w2h = np.ascontiguousarray((0.1 * W2.astype(np.float32)).transpose(1, 0, 2))
    embed = np.ascontiguousarray(embed.astype(np.float32))
    tm = token_mask.astype(np.float32)

    in_maps = []
    for c in range(N_CORES):
        xc = xf[c * T : (c + 1) * T]
        idx_c = np.ascontiguousarray(xc.reshape(T // P, P).T)  # [p, t]
        maskt_c = np.ascontiguousarray(tm[xc].T)               # [16, T]
        in_maps.append(
            {
                "idx": idx_c,
                "embed": embed,
                "w1": w1h,
                "w2": w2h,
                "maskt": maskt_c,
            }
        )
    return in_maps


def get_program():
    if "nc" not in _CACHE:
        _CACHE["nc"] = _build_program()
    return _CACHE["nc"]


def kernel(x, embed, W1, W2, token_mask):
    nc = get_program()
    in_maps = _prep_inputs(x, embed, W1, W2, token_mask)
    res = run_bass_kernel_spmd(nc, in_maps, core_ids=list(range(N_CORES)))
    out = np.concatenate([r["out"] for r in res.results], axis=0)
    return out.reshape(B, S, D)


# revision 12
# speedup vs baseline: 1.0548x; 1.0548x over previous
"""Trainium2 Bass kernel for ExpandFormerV16 (masked multi-domain MLP over embeddings).

Reference computation:
    h    = embed[x]                                   # [B,S,512]
    mask = token_mask[x]                              # [B,S,16]
    act  = gelu(einsum('bsD,nDd->bsnd', h, W1))       # exact (erf) gelu
    corr = 0.1 * einsum('bsnd,bsn,ndD->bsD', act, mask, W2)
    out  = h + corr

Strategy: data-parallel over the 16384 tokens -> 2048 tokens per core on 8
cores; embedding table and domain weights replicated. Per core:
  - indirect-DMA gather of embed rows (h stays exact fp32 for the final add)
  - PE transpose h -> hT [D, tok]
  - GEMM1 (f32r): actT[n] = W1[n].T @ hT, K=512 accumulated in PSUM
  - exact-erf Gelu on ACT engine, mask multiply on DVE (mask rows broadcast
    across partitions with a stride-0-partition DMA)
  - GEMM2 (f32r): corr[tok,:] += actmT[n].T @ (0.1*W2[n]) over all 16 domains
  - DVE add h + corr, DMA out
"""

import ml_dtypes
import numpy as np

import concourse.bacc as bacc
import concourse.bass as bass
import concourse.tile as tile
from concourse import mybir
from concourse.bass import IndirectOffsetOnAxis
from concourse.bass_utils import run_bass_kernel_spmd
from concourse.masks import make_identity

# Problem shapes (hardcoded per contest contract)
VOCAB, D, ND, DD = 32000, 512, 16, 128
B, S = 8, 2048
N_CORES = 8
T = (B * S) // N_CORES          # tokens per core = 2048
P = 128                         # partitions
KCH = D // P                    # 4 contraction chunks of 128
TBLK = 512                      # tokens per processing block (PSUM free dim)
NBLK = T // TBLK                # 4 blocks per core
JT = TBLK // P                  # 4 token-tiles of 128 per block

F32 = mybir.dt.float32
F32R = mybir.dt.float32r
I32 = mybir.dt.int32

_CACHE: dict = {}


def _build_program():
    nc = bacc.Bacc(
        "TRN2",
        target_bir_lowering=False,
        debug=False,
        enable_asserts=False,
        num_devices=N_CORES,
    )

    idx_d = nc.dram_tensor("idx", [P, T // P], I32, kind="ExternalInput")
    embed_d = nc.dram_tensor("embed", [VOCAB, D], F32, kind="ExternalInput")
    w1_d = nc.dram_tensor("w1", [P, KCH, ND, DD], F32R, kind="ExternalInput")
    w2_d = nc.dram_tensor("w2", [P, ND, D], F32R, kind="ExternalInput")
    maskt_d = nc.dram_tensor("maskt", [ND, T], mybir.dt.bfloat16, kind="ExternalInput")
    out_d = nc.dram_tensor("out", [T, D], F32, kind="ExternalOutput")

    with tile.TileContext(nc) as tc:
        with (
            tc.tile_pool(name="consts", bufs=1) as consts,
            tc.tile_pool(name="hpool", bufs=2) as hpool,
            tc.tile_pool(name="htpool", bufs=2) as htpool,
            tc.tile_pool(name="gpool", bufs=3) as gpool,
            tc.tile_pool(name="mpool", bufs=3) as mpool,
            tc.tile_pool(name="ampool", bufs=2) as ampool,
            tc.tile_pool(name="opool", bufs=3) as opool,
            tc.tile_pool(name="tpsum", bufs=2, space="PSUM") as tpsum,
            tc.tile_pool(name="apsum", bufs=3, space="PSUM") as apsum,
            tc.tile_pool(name="cpsum", bufs=3, space="PSUM") as cpsum,
        ):
            # --- constants ---
            idx_sb = consts.tile([P, T // P], I32)
            nc.sync.dma_start(idx_sb[:], idx_d.ap())
            w1_sb = consts.tile([P, KCH, ND, DD], F32R)
            nc.sync.dma_start(w1_sb[:], w1_d.ap())
            w2_sb = consts.tile([P, ND, D], F32R)
            nc.sync.dma_start(w2_sb[:], w2_d.ap())
            ident = consts.tile([P, P], F32)
            make_identity(nc, ident[:])

            for blk in range(NBLK):
                # --- gather embed rows for this block's 4 token-tiles ---
                h_blk = hpool.tile([P, JT, D], F32)
                for j in range(JT):
                    t = blk * JT + j
                    nc.gpsimd.indirect_dma_start(
                        out=h_blk[:, j, :],
                        out_offset=None,
                        in_=embed_d.ap(),
                        in_offset=IndirectOffsetOnAxis(
                            ap=idx_sb[:, t : t + 1], axis=0
                        ),
                    )

                # --- transpose h [tok, D] -> hT [D, tok] via PE ---
                hT_blk = htpool.tile([P, KCH, TBLK], F32R)
                for j in range(JT):
                    for k in range(KCH):
                        tp = tpsum.tile([P, P], F32)
                        nc.tensor.transpose(
                            tp[:], h_blk[:, j, k * P : (k + 1) * P], ident[:]
                        )
                        nc.vector.tensor_copy(
                            hT_blk[:, k, j * P : (j + 1) * P], tp[:]
                        )

                # --- GEMM1 + gelu + mask per domain ---
                actm_blk = ampool.tile([P, ND, TBLK], F32R)
                for n in range(ND):
                    act_ps = apsum.tile([P, TBLK], F32)
                    for k in range(KCH):
                        nc.tensor.matmul(
                            act_ps[:],
                            lhsT=w1_sb[:, k, n, :],
                            rhs=hT_blk[:, k, :],
                            start=(k == 0),
                            stop=(k == KCH - 1),
                        )
                    actg = gpool.tile([P, TBLK], F32)
                    nc.scalar.activation(
                        actg[:], act_ps[:], mybir.ActivationFunctionType.Gelu
                    )
                    # mask row [1, TBLK] broadcast to all 128 partitions via
                    # stride-0 partition DMA (groupnorm bias pattern)
                    m_tile = mpool.tile([P, TBLK], mybir.dt.bfloat16)
                    m_src = bass.AP(
                        tensor=maskt_d.ap().tensor,
                        offset=n * T + blk * TBLK,
                        ap=[[0, P], [1, TBLK]],
                    )
                    nc.sync.dma_start(out=m_tile[:], in_=m_src)
                    nc.vector.tensor_mul(actm_blk[:, n, :], actg[:], m_tile[:])

                # --- GEMM2: corr[tok, D] accumulated over domains ---
                for j in range(JT):
                    corr_ps = cpsum.tile([P, D], F32)
                    for n in range(ND):
                        nc.tensor.matmul(
                            corr_ps[:],
                            lhsT=actm_blk[:, n, j * P : (j + 1) * P],
                            rhs=w2_sb[:, n, :],
                            start=(n == 0),
                            stop=(n == ND - 1),
                        )
                    out_sb = opool.tile([P, D], F32)
                    nc.vector.tensor_add(out_sb[:], corr_ps[:], h_blk[:, j, :])
                    row0 = (blk * JT + j) * P
                    nc.sync.dma_start(
                        out=out_d.ap()[row0 : row0 + P, :], in_=out_sb[:]
                    )

    nc.compile()
    return nc


def _prep_inputs(x, embed, W1, W2, token_mask):
    """Host-side shard + layout prep. Returns per-core in_maps."""
    xf = np.ascontiguousarray(x.reshape(-1).astype(np.int32))
    w1h = np.ascontiguousarray(
        W1.astype(np.float32)
        .transpose(1, 0, 2)          # [512, 16, 128]
        .reshape(KCH, P, ND, DD)     # [k, p, n, d]
        .transpose(1, 0, 2, 3)       # [p, k, n, d]
    )
    w2h = np.ascontiguousarray((0.1 * W2.astype(np.float32)).transpose(1, 0, 2))
    embed = np.ascontiguousarray(embed.astype(np.float32))
    tm = token_mask.astype(np.float32)

    in_maps = []
    for c in range(N_CORES):
        xc = xf[c * T : (c + 1) * T]
        idx_c = np.ascontiguousarray(xc.reshape(T // P, P).T)  # [p, t]
        maskt_c = np.ascontiguousarray(tm[xc].T).astype(ml_dtypes.bfloat16)  # [16, T]
        in_maps.append(
            {
                "idx": idx_c,
                "embed": embed,
                "w1": w1h,
                "w2": w2h,
                "maskt": maskt_c,
            }
        )
    return in_maps


def get_program():
    if "nc" not in _CACHE:
        _CACHE["nc"] = _build_program()
    return _CACHE["nc"]


def kernel(x, embed, W1, W2, token_mask):
    nc = get_program()
    in_maps = _prep_inputs(x, embed, W1, W2, token_mask)
    res = run_bass_kernel_spmd(nc, in_maps, core_ids=list(range(N_CORES)))
    out = np.concatenate([r["out"] for r in res.results], axis=0)
    return out.reshape(B, S, D)


# revision 50
# speedup vs baseline: 1.3632x; 1.2924x over previous
"""Trainium2 Bass kernel for ExpandFormerV16 (masked multi-domain MLP over embeddings).

Reference computation:
    h    = embed[x]                                   # [B,S,512]
    mask = token_mask[x]                              # [B,S,16]
    act  = gelu(einsum('bsD,nDd->bsnd', h, W1))       # exact (erf) gelu
    corr = 0.1 * einsum('bsnd,bsn,ndD->bsD', act, mask, W2)
    out  = h + corr

Strategy: data-parallel over the 16384 tokens -> 2048 tokens per core on 8
cores; embedding table and domain weights replicated. Per core:
  - indirect-DMA gather of embed rows (h stays exact fp32 for the final add)
  - PE transpose h -> hT [D, tok]
  - GEMM1 (f32r, 1 cyc/row): actT[n] = W1[n].T @ hT, K=512 accumulated in PSUM
  - exact-erf Gelu on ACT engine -> bf16, mask multiply on DVE (mask rows
    broadcast across partitions with a stride-0-partition DMA, one per block)
  - GEMM2 (bf16): corr[tok,:] += actmT[n].T @ (0.1*W2[n]) over all 16 domains
  - DVE add h + corr (fp32), DMA out

The mask/GEMM2 path runs in bf16: corr is ~0.3% of |out|, so bf16 rounding
there contributes ~1e-5 relative error to the output while halving SBUF and
DMA traffic (PE rate is identical to f32r at N=512).
"""

import ml_dtypes
import numpy as np

import concourse.bacc as bacc
import concourse.bass as bass
import concourse.tile as tile
from concourse import mybir
from concourse.bass import IndirectOffsetOnAxis
from concourse.bass_utils import run_bass_kernel_spmd

# Problem shapes (hardcoded per contest contract)
VOCAB, D, ND, DD = 32000, 512, 16, 128
B, S = 8, 2048
N_CORES = 8
T = (B * S) // N_CORES          # tokens per core = 2048
P = 128                         # partitions
KCH = D // P                    # 4 contraction chunks of 128
TBLK = 512                      # tokens per processing block (PSUM free dim)
NBLK = T // TBLK                # 4 blocks per core
JT = TBLK // P                  # 4 token-tiles of 128 per block
WCH = 4                         # domains per weight-load DMA

F32 = mybir.dt.float32
F32R = mybir.dt.float32r
BF16 = mybir.dt.bfloat16
I32 = mybir.dt.int32

_CACHE: dict = {}


def _build_program():
    nc = bacc.Bacc(
        "TRN2",
        target_bir_lowering=False,
        debug=False,
        enable_asserts=False,
        num_devices=N_CORES,
    )

    idx_d = nc.dram_tensor("idx", [P, T // P], I32, kind="ExternalInput")
    # idx16[p, b, c] = x[b*TBLK + c*16 + p%16] (column-major 16-wrap per
    # block, replicated over the 8 gpsimd cores) — dma_gather's index layout
    idx16_d = nc.dram_tensor("idx16", [P, NBLK * 2, TBLK // 32], mybir.dt.int16, kind="ExternalInput")
    embed_d = nc.dram_tensor("embed", [VOCAB, D], F32, kind="ExternalInput")
    embed16_d = nc.dram_tensor("embed16", [VOCAB, D], BF16, kind="ExternalInput")
    w1_d = nc.dram_tensor("w1", [P, ND, KCH, DD], BF16, kind="ExternalInput")
    w2_d = nc.dram_tensor("w2", [P, ND, D], BF16, kind="ExternalInput")
    maskt_d = nc.dram_tensor("maskt", [ND, T], BF16, kind="ExternalInput")
    out_d = nc.dram_tensor("out", [T, D], F32, kind="ExternalOutput")

    with tile.TileContext(nc) as tc:
        with (
            tc.tile_pool(name="consts", bufs=1) as consts,
            tc.tile_pool(name="hpool", bufs=2) as hpool,
            tc.tile_pool(name="htpool", bufs=2) as htpool,
            tc.tile_pool(name="gpool", bufs=3) as gpool,
            tc.tile_pool(name="mpool", bufs=2) as mpool,
            tc.tile_pool(name="ampool", bufs=2) as ampool,
            tc.tile_pool(name="opool", bufs=2) as opool,
            tc.tile_pool(name="apsum", bufs=4, space="PSUM") as apsum,
            tc.tile_pool(name="cpsum", bufs=4, space="PSUM") as cpsum,
        ):
            # --- constants ---
            idx16_sb = consts.tile([P, NBLK * 2, TBLK // 32], mybir.dt.int16)
            nc.sync.dma_start(idx16_sb[:], idx16_d.ap())
            idx_sb = consts.tile([P, T // P], I32)
            nc.sync.dma_start(idx_sb[:], idx_d.ap())

            def gather_block(blk):
                h_blk = hpool.tile([P, JT, D], F32, tag="h_blk")
                for j in range(JT):
                    t = blk * JT + j
                    nc.gpsimd.indirect_dma_start(
                        out=h_blk[:, j, :],
                        out_offset=None,
                        in_=embed_d.ap(),
                        in_offset=IndirectOffsetOnAxis(
                            ap=idx_sb[:, t : t + 1], axis=0
                        ),
                    )
                return h_blk

            def gather_t_block(blk):
                # transposed bf16 gather: hT_blk[p, k, t] = embed16[x[t], 128k+p]
                hT_blk = htpool.tile([P, KCH, TBLK], BF16, tag="hT_blk")
                nc.gpsimd.dma_gather(
                    hT_blk[:],
                    embed16_d.ap(),
                    idx16_sb[:, blk * 2 : blk * 2 + 2, :],
                    TBLK,
                    TBLK,
                    D,
                    transpose=True,
                )
                return hT_blk

            def load_mask_block(blk):
                # all 16 domain rows for this block, each broadcast to 128
                # partitions via stride-0 partition dim (groupnorm bias
                # pattern); chunked 4 domains per DMA to interleave with
                # other transfers
                m_blk = mpool.tile([P, ND, TBLK], BF16, tag="m_blk")
                for c in range(0, ND, WCH):
                    m_src = bass.AP(
                        tensor=maskt_d.ap().tensor,
                        offset=c * T + blk * TBLK,
                        ap=[[0, P], [T, WCH], [1, TBLK]],
                    )
                    nc.sync.dma_start(out=m_blk[:, c : c + WCH, :], in_=m_src)
                return m_blk

            # kick off block 0 gathers + transposes before the bulk weight
            # loads so the PE starts immediately and the gathers get the DMA
            # engines first
            hT_cur = gather_t_block(0)

            # weights, loaded in chunks so GEMM1(n) unblocks early; the first
            # chunk is a single domain so the PE's first Ldweights fires asap
            w1_sb = consts.tile([P, ND, KCH, DD], BF16)
            w2_sb = consts.tile([P, ND, D], BF16)
            w1_chunks = [(0, 1), (1, 3)] + [(c, WCH) for c in range(WCH, ND, WCH)]
            for c, w in w1_chunks:
                nc.sync.dma_start(
                    w1_sb[:, c : c + w, :, :], w1_d.ap()[:, c : c + w, :, :]
                )
            for c in range(0, ND, WCH):
                nc.sync.dma_start(
                    w2_sb[:, c : c + WCH, :], w2_d.ap()[:, c : c + WCH, :]
                )
            m_cur = load_mask_block(0)
            h_cur = gather_block(0)

            for blk in range(NBLK):
                h_blk, hT_blk, m_blk = h_cur, hT_cur, m_cur

                # --- GEMM1 + gelu + mask per domain ---
                actm_blk = ampool.tile([P, ND, TBLK], BF16)
                for n in range(ND):
                    act_ps = apsum.tile([P, TBLK], F32)
                    for k in range(KCH):
                        nc.tensor.matmul(
                            act_ps[:],
                            lhsT=w1_sb[:, n, k, :],
                            rhs=hT_blk[:, k, :],
                            start=(k == 0),
                            stop=(k == KCH - 1),
                        )
                    actg = gpool.tile([P, TBLK], BF16)
                    nc.scalar.activation(
                        actg[:], act_ps[:], mybir.ActivationFunctionType.Gelu
                    )
                    nc.vector.tensor_mul(
                        actm_blk[:, n, :], actg[:], m_blk[:, n, :]
                    )

                # prefetch next block's inputs (hT first: it gates GEMM1)
                if blk + 1 < NBLK:
                    hT_cur = gather_t_block(blk + 1)
                    m_cur = load_mask_block(blk + 1)
                    h_cur = gather_block(blk + 1)

                # --- GEMM2: corr[tok, D] accumulated over domains ---
                for j in range(JT):
                    corr_ps = cpsum.tile([P, D], F32)
                    for n in range(ND):
                        nc.tensor.matmul(
                            corr_ps[:],
                            lhsT=actm_blk[:, n, j * P : (j + 1) * P],
                            rhs=w2_sb[:, n, :],
                            start=(n == 0),
                            stop=(n == ND - 1),
                        )
                    out_sb = opool.tile([P, D], F32, tag="out_sb")
                    nc.vector.tensor_add(out_sb[:], corr_ps[:], h_blk[:, j, :])
                    row0 = (blk * JT + j) * P
                    nc.sync.dma_start(
                        out=out_d.ap()[row0 : row0 + P, :], in_=out_sb[:]
                    )

    nc.compile()
    return nc


def _prep_inputs(x, embed, W1, W2, token_mask):
    """Host-side shard + layout prep. Returns per-core in_maps."""
    xf = np.ascontiguousarray(x.reshape(-1).astype(np.int32))
    w1h = np.ascontiguousarray(
        W1.astype(np.float32)
        .reshape(ND, KCH, P, DD)     # [n, k, p, d]
        .transpose(2, 0, 1, 3)       # [p, n, k, d]
    ).astype(ml_dtypes.bfloat16)
    w2h = np.ascontiguousarray(
        (0.1 * W2.astype(np.float32)).transpose(1, 0, 2)
    ).astype(ml_dtypes.bfloat16)
    embed = np.ascontiguousarray(embed.astype(np.float32))
    embed16 = embed.astype(ml_dtypes.bfloat16)
    tm = token_mask.astype(np.float32)

    in_maps = []
    for c in range(N_CORES):
        xc = xf[c * T : (c + 1) * T]
        idx_c = np.ascontiguousarray(xc.reshape(T // P, P).T)  # [p, t]
        # dma_gather index layout: [16-wrap column-major, tiled to 128 rows],
        # one gather unit per half-block of 256 tokens
        idx16_c = np.ascontiguousarray(
            np.tile(
                xc.astype(np.int16)
                .reshape(NBLK * 2, TBLK // 32, 16)
                .transpose(0, 2, 1),     # [g, 16, TBLK//32]
                (1, 8, 1),               # -> [g, 128, TBLK//32]
            ).transpose(1, 0, 2)         # -> [128, g, TBLK//32]
        )
        maskt_c = np.ascontiguousarray(tm[xc].T).astype(ml_dtypes.bfloat16)
        in_maps.append(
            {
                "idx": idx_c,
                "idx16": idx16_c,
                "embed": embed,
                "embed16": embed16,
                "w1": w1h,
                "w2": w2h,
                "maskt": maskt_c,
            }
        )
    return in_maps


def get_program():
    if "nc" not in _CACHE:
        _CACHE["nc"] = _build_program()
    return _CACHE["nc"]


def kernel(x, embed, W1, W2, token_mask):
    nc = get_program()
    in_maps = _prep_inputs(x, embed, W1, W2, token_mask)
    res = run_bass_kernel_spmd(nc, in_maps, core_ids=list(range(N_CORES)))
    out = np.concatenate([r["out"] for r in res.results], axis=0)
    return out.reshape(B, S, D)
